# revision 27
# baseline (speedup 1.0000x reference)
"""Trainium2 Bass kernel for a binarized (1w1a) BasicBlock:

    out1 = hardtanh(BN1(binconv(x, w1)))          # BN in training mode (batch stats)
    out  = hardtanh(BN2(binconv(out1, w2)) + x)   # identity shortcut

binconv(x, w) = conv3x3(sign(x), sign(w), pad=1) * (SCALE / K)

Sharding: data-parallel over batch (4 images per core on 8 cores), weights
replicated.  BN batch statistics (per-channel sum and sum-of-squares) are
combined with tiny cross-core AllReduces.

Implementation notes:
  - sign() values (+-1, 0) are exact in fp8, and the 3x3x256 conv
    accumulates integers |S| <= 2304 in fp32 PSUM, so the convolutions are
    bit-exact at fp8 TensorE rate.  The SCALE/K factor commutes through
    BatchNorm and is folded into eps:  eps_eff = eps / (SCALE/K)^2.
  - conv3x3 = 9 shifted matmuls accumulated in PSUM, with fp8 DoubleRow
    contracting both 128-channel blocks per pass.  Activations live in SBUF
    as [128 ch-pair, 2, n, 58, 64] zero-padded images, so every shift is an
    access-pattern offset.  Each matmul reads 7 padded rows contiguously
    (7 x 64 = 448 columns); the junk columns between output rows are simply
    never read back.  (A 2-D rhs access pattern that skips the padding was
    measured SLOWER: the PE pays a per-row AP overhead.)
  - Weights are pre-transposed on the host to the exact SBUF layout
    ([ci%128, dy, dx, ci//128, co]) so the weight DMA is contiguous.
  - x is loaded from HBM once, on two parallel DMA queues (SP carries the
    first channel block, the Activation queue the second); the staged fp32
    tile feeds both the sign activation (ScalarE -> fp8 conv input) and an
    fp16 residual copy (VectorE) kept resident for the final shortcut add.
  - BN statistics are AllReduced per image: the first three chunks overlap
    the conv (and absorb cross-core launch skew early); only the last
    image's collective latency is exposed at the phase boundary.  All the
    collective plumbing lives on the gpsimd queue so it never blocks the
    x-load or compute queues.
  - The final phase runs in fp16 (exact conv integers + ~1e-3 rounding,
    well inside tolerance), load-balanced: affine on ScalarE (6 blocks) /
    VectorE (2), residual add on VectorE, clip on VectorE / Pool, output
    DMA'd at fp16 on two queues (host converts to fp32).
"""

import numpy as np
import ml_dtypes

import concourse.bass as bass
import concourse.tile as tile
from concourse import bacc, mybir
from concourse import bass_utils

N_CORES = 8
N, C, H, W = 32, 256, 56, 56
NL = N // N_CORES          # images per core
HP = H + 2                 # padded rows (58)
WP = 64                    # padded row pitch: keeps DoubleRow offsets 16B-aligned
IMG = HP * WP              # padded image elements
CB = C // 128              # channel blocks (2)
HT = 7                     # output rows per tile
N_HT = H // HT             # 8 tiles per image
FREE = HT * WP             # 448: 7 padded rows read contiguously
SCALE, K = 1.0, 2
EPS = 1e-5
ALPHA = SCALE / K
EPS_EFF = EPS / (ALPHA * ALPHA)
M_TOT = float(N * H * W)   # BN reduction count (global batch)
HW_ = H * W

F32 = mybir.dt.float32
F16 = mybir.dt.float16
FP8 = mybir.dt.float8e4
NP_FP8 = ml_dtypes.float8_e4m3
AF = mybir.ActivationFunctionType
ALU = mybir.AluOpType
DR = mybir.MatmulPerfMode.DoubleRow

_CACHE = {}


def _conv_img(nc, psum, xbf, wt, y16, recs, n):
    """One binarized conv3x3 over local image n + its stats records."""
    for ht in range(N_HT):
        h0 = ht * HT
        for cob in range(CB):
            # out[h0+r, w] accumulates at pt column r*WP + w + 2 for all
            # nine (dy, dx): the dx shift is applied to the PSUM window so
            # every rhs read stays 16B-aligned (DoubleRow requirement).
            pt = psum.tile([128, FREE + 2], F32, tag="pt", name="pt")
            k = 0
            for dy in range(3):
                for dx in range(3):
                    off = n * IMG + (h0 + dy) * WP
                    nc.tensor.matmul(
                        pt[:, 2 - dx:FREE + 2 - dx],
                        wt[:, dy, dx, :, cob * 128:(cob + 1) * 128],
                        xbf[:, :, off:off + FREE],
                        start=(k == 0),
                        stop=(k == 8),
                        perf_mode=DR,
                    )
                    k += 1
            ptv = pt[:, 2:FREE + 2].rearrange("p (a b) -> p a b", b=WP)
            # fp32 PSUM -> exact integers in fp16
            nc.scalar.activation(
                out=y16[cob][:, n, h0:h0 + HT, :], in_=ptv[:, :, 0:W],
                func=AF.Copy)
            # one Welford record per tile (VectorE, contiguous fp16 read;
            # bn_stats is limited to 512 elements per record)
            yfl = y16[cob][:, n, :, :].rearrange("p a b -> p (a b)")
            nc.vector.bn_stats(
                out=recs[cob][:, n, ht, :],
                in_=yfl[:, h0 * W:(h0 + HT) * W])


def _stats_partial(nc, pools, recs, lo, hi, tagp):
    """bn_aggr records of images [lo, hi) into local (sum, sumsq) [128, 4]."""
    small = pools["small"]
    m_loc = float((hi - lo) * HW_)
    st = small.tile([128, 4], F32, tag=f"st{tagp}", name=f"st{tagp}")
    for cob in range(CB):
        mv = small.tile([128, 2], F32, tag=f"mv{tagp}{cob}", name=f"mv{tagp}{cob}")
        rv = recs[cob][:, lo:hi, :, :].rearrange("p n t s -> p (n t) s")
        nc.vector.bn_aggr(out=mv[:], in_=rv)
        msq = small.tile([128, 1], F32, tag=f"sq{tagp}{cob}", name=f"sq{tagp}{cob}")
        nc.vector.tensor_scalar_mul(st[:, 2 * cob:2 * cob + 1], mv[:, 0:1], m_loc)
        nc.vector.tensor_mul(msq[:], mv[:, 0:1], mv[:, 0:1])
        nc.vector.tensor_add(msq[:], msq[:], mv[:, 1:2])
        nc.vector.tensor_scalar_mul(st[:, 2 * cob + 1:2 * cob + 2], msq[:], m_loc)
    return st


def _stats_allreduce(nc, pools, st_a, st_b, tagp):
    """AllReduce st_a + st_b across the 8 cores.  One collective per conv:
    the exposed boundary cost is launch-skew + one collective latency no
    matter how the stats are chunked.  st_a covers images 0-2 and is
    aggregated while the conv still runs; only st_b's short chain sits on
    the critical path.  All plumbing rides the gpsimd queue."""
    small, dram = pools["small"], pools["dram"]
    rg = [list(range(N_CORES))]
    st = small.tile([128, 4], F32, tag=f"stt{tagp}", name=f"stt{tagp}")
    nc.vector.tensor_add(st[:], st_a[:], st_b[:])
    d_in = dram.tile([128, 4], F32, tag=f"din{tagp}", name=f"din{tagp}")
    d_out = dram.tile([128, 4], F32, tag=f"dout{tagp}", name=f"dout{tagp}")
    nc.gpsimd.dma_start(out=d_in[:], in_=st[:])
    nc.gpsimd.collective_compute(
        "AllReduce", ALU.add, replica_groups=rg,
        ins=[d_in.opt()], outs=[d_out.opt()],
    )
    gsum = small.tile([128, 4], F32, tag=f"gs{tagp}", name=f"gs{tagp}")
    nc.gpsimd.dma_start(out=gsum[:], in_=d_out[:])
    return gsum


def _bn_affine(nc, pools, gstats, gb, g_col, b_col, a_out, b_out):
    """Per-channel-block A/B:  A = g * rsqrt(var + eps_eff),  B = b - mean * A.

    gstats: [128, 4] globally-reduced (sum, sumsq) per channel block
    """
    small = pools["small"]
    epst = pools["epst"]
    for cob in range(CB):
        mean = small.tile([128, 1], F32, tag=f"mean{cob}", name=f"mean{cob}")
        ex2 = small.tile([128, 1], F32, tag=f"ex2{cob}", name=f"ex2{cob}")
        msq = small.tile([128, 1], F32, tag=f"msq{cob}", name=f"msq{cob}")
        var = small.tile([128, 1], F32, tag=f"var{cob}", name=f"var{cob}")
        rstd = small.tile([128, 1], F32, tag=f"rstd{cob}", name=f"rstd{cob}")
        nc.vector.tensor_scalar_mul(mean[:], gstats[:, 2 * cob:2 * cob + 1], 1.0 / M_TOT)
        nc.vector.tensor_scalar_mul(ex2[:], gstats[:, 2 * cob + 1:2 * cob + 2], 1.0 / M_TOT)
        # var = ex2 - mean^2
        nc.vector.tensor_mul(msq[:], mean[:], mean[:])
        nc.vector.tensor_sub(var[:], ex2[:], msq[:])
        # rstd = 1 / sqrt(var + eps_eff)
        nc.scalar.activation(out=rstd[:], in_=var[:], func=AF.Sqrt, bias=epst[:])
        nc.vector.reciprocal(rstd[:], rstd[:])
        # A = g * rstd ; B = b - mean * A
        nc.vector.tensor_mul(a_out[cob][:], gb[:, g_col + cob:g_col + cob + 1], rstd[:])
        nc.vector.tensor_mul(mean[:], mean[:], a_out[cob][:])
        nc.vector.tensor_sub(b_out[cob][:], gb[:, b_col + cob:b_col + cob + 1], mean[:])


def build():
    """Build + compile the per-core Bass program (SPMD, 8 cores)."""
    nc = bacc.Bacc("TRN2", target_bir_lowering=False, debug=False,
                   num_devices=N_CORES)

    x_in = nc.dram_tensor("x16", [NL, C, H, W], F16, kind="ExternalInput").ap()
    w1_in = nc.dram_tensor("w1t", [128, 3, 3, 2, C], FP8, kind="ExternalInput").ap()
    w2_in = nc.dram_tensor("w2t", [128, 3, 3, 2, C], FP8, kind="ExternalInput").ap()
    gb_in = nc.dram_tensor("gb", [128, 8], F32, kind="ExternalInput").ap()
    out_d = nc.dram_tensor("out", [NL, C, H, W], F16, kind="ExternalOutput").ap()

    rg = [list(range(N_CORES))]

    with tile.TileContext(nc) as tc:
        import contextlib
        with contextlib.ExitStack() as ctx:
            consts = ctx.enter_context(tc.tile_pool(name="consts", bufs=1))
            xbp = ctx.enter_context(tc.tile_pool(name="xbp", bufs=1))
            y16p = ctx.enter_context(tc.tile_pool(name="y16p", bufs=1))
            xsp = ctx.enter_context(tc.tile_pool(name="xsp", bufs=1))
            statp = ctx.enter_context(tc.tile_pool(name="statp", bufs=1))
            small = ctx.enter_context(tc.tile_pool(name="small", bufs=1))
            psum = ctx.enter_context(tc.tile_pool(name="psum", bufs=6, space="PSUM"))
            dram = ctx.enter_context(tc.tile_pool(name="dram", bufs=1, space="DRAM"))
            youtp = ctx.enter_context(tc.tile_pool(name="youtp", bufs=8))
            pools = {"small": small, "dram": dram}

            # ---- phase-0 head start: x arrives as fp16 (host-converted,
            # half the HBM traffic) into per-image persistent tiles that
            # double as the residual for the final add.  Image 0's two
            # blocks load on parallel queues (critical path to the first
            # matmul); later images go serially on the SP queue so the DMA
            # traffic is spread thin and does not steal SBUF bandwidth from
            # the PE's rhs reads.
            def phase0(n):
                xs_pair = []
                for cib in range(CB):
                    xs = xsp.tile([128, H, W], F16, tag=f"xs{n}_{cib}",
                                  name=f"xs{n}_{cib}")
                    dma_q = nc.scalar if (n == 0 and cib == 1) else nc.sync
                    dma_q.dma_start(
                        out=xs[:], in_=x_in[n, cib * 128:(cib + 1) * 128, :, :])
                    xs_pair.append(xs)
                return xs_pair

            xs0 = phase0(0)

            # ---- w1 first on the gpsimd queue: the first matmul needs it
            w1t = consts.tile([128, 3, 3, 2, C], FP8, tag="w1t", name="w1t")
            nc.gpsimd.dma_start(out=w1t[:], in_=w1_in[:])

            epst = small.tile([128, 1], F32, tag="epst", name="epst")
            nc.vector.memset(epst[:], EPS_EFF)
            pools["epst"] = epst

            # ---- dummy AllReduce: absorb first-collective setup cost
            dzero = small.tile([128, 1], F32, tag="dzero", name="dzero")
            nc.vector.memset(dzero[:], 0.0)
            d_in0 = dram.tile([128, 1], F32, tag="d_in0", name="d_in0")
            d_out0 = dram.tile([128, 1], F32, tag="d_out0", name="d_out0")
            nc.gpsimd.dma_start(out=d_in0[:], in_=dzero[:])
            nc.gpsimd.collective_compute(
                "AllReduce", ALU.add, replica_groups=rg,
                ins=[d_in0.opt()], outs=[d_out0.opt()],
            )

            # ---- remaining constants
            w2t = consts.tile([128, 3, 3, 2, C], FP8, tag="w2t", name="w2t")
            nc.gpsimd.dma_start(out=w2t[:], in_=w2_in[:])
            gb = consts.tile([128, 8], F32, tag="gb", name="gb")
            nc.gpsimd.dma_start(out=gb[:], in_=gb_in[:])

            # ---- padded binarized activations (reused: conv1 input, then conv2
            # input).  Rows padded to a 64B pitch so DoubleRow rhs offsets stay
            # 16B-aligned; block stride NL*IMG is 16B-aligned too.
            blk = NL * IMG
            xb = xbp.tile([128, CB, blk], FP8, tag="xb", name="xb")
            xbf = xb[:]
            xbi = [xb[:, cib, :].rearrange(
                "p (n a b) -> p n a b", a=HP, b=WP) for cib in range(CB)]
            # zero only the halo borders + pitch padding (interior is always
            # overwritten by the sign activations before it is read)
            for cib in range(CB):
                nc.vector.memset(xbi[cib][:, :, 0, :], 0.0)
                nc.vector.memset(xbi[cib][:, :, H + 1, :], 0.0)
                nc.vector.memset(xbi[cib][:, :, 1:H + 1, 0:1], 0.0)
                nc.vector.memset(xbi[cib][:, :, 1:H + 1, W + 1:WP], 0.0)

            # ---- conv outputs as exact integers (reused for conv1 then conv2)
            y16 = [y16p.tile([128, NL, H, W], F16, tag=f"y16_{cob}", name=f"y16_{cob}")
                   for cob in range(CB)]

            # ---- bn_stats records (one per row-tile)
            r1c = [statp.tile([128, NL, N_HT, 6], F32, tag=f"r1c{c}", name=f"r1c{c}")
                   for c in range(CB)]
            r2c = [statp.tile([128, NL, N_HT, 6], F32, tag=f"r2c{c}", name=f"r2c{c}")
                   for c in range(CB)]

            # ---- binarize (ScalarE): sign(x) from the fp16 staged tiles
            def binarize(n, xs_pair):
                for cib in range(CB):
                    nc.scalar.activation(
                        out=xbi[cib][:, n, 1:H + 1, 1:W + 1], in_=xs_pair[cib][:],
                        func=AF.Sign)

            # ---- conv1 (loads/signs lead the matmuls)
            xst = [xs0]
            binarize(0, xs0)
            xst.append(phase0(1))
            binarize(1, xst[1])
            _conv_img(nc, psum, xbf, w1t, y16, r1c, 0)
            xst.append(phase0(2))
            binarize(2, xst[2])
            _conv_img(nc, psum, xbf, w1t, y16, r1c, 1)
            xst.append(phase0(3))
            binarize(3, xst[3])
            _conv_img(nc, psum, xbf, w1t, y16, r1c, 2)
            st1a = _stats_partial(nc, pools, r1c, 0, 3, "1a")
            _conv_img(nc, psum, xbf, w1t, y16, r1c, 3)
            st1b = _stats_partial(nc, pools, r1c, 3, 4, "1b")

            gstats1 = _stats_allreduce(nc, pools, st1a, st1b, "1")
            a1 = [small.tile([128, 1], F32, tag=f"a1_{c}", name=f"a1_{c}") for c in range(CB)]
            b1 = [small.tile([128, 1], F32, tag=f"b1_{c}", name=f"b1_{c}") for c in range(CB)]
            _bn_affine(nc, pools, gstats1, gb, g_col=0, b_col=2, a_out=a1, b_out=b1)

            # ---- phase 2 + conv2, pipelined per image:
            # out1 = sign(A1 * y1 + B1) written back into the padded buffers.
            # Image 0's signs are split into row halves so conv2's first
            # tiles start after half the sign latency.
            def sign2(n, halves=False):
                rr = [(0, H // 2), (H // 2, H)] if halves else [(0, H)]
                for r0, r1 in rr:
                    for cob in range(CB):
                        nc.scalar.activation(
                            out=xbi[cob][:, n, 1 + r0:1 + r1, 1:W + 1],
                            in_=y16[cob][:, n, r0:r1, :],
                            func=AF.Sign,
                            scale=a1[cob][:],
                            bias=b1[cob][:],
                        )

            sign2(0, halves=True)
            sign2(1)
            _conv_img(nc, psum, xbf, w2t, y16, r2c, 0)
            sign2(2)
            _conv_img(nc, psum, xbf, w2t, y16, r2c, 1)
            sign2(3)
            _conv_img(nc, psum, xbf, w2t, y16, r2c, 2)
            st2a = _stats_partial(nc, pools, r2c, 0, 3, "2a")
            _conv_img(nc, psum, xbf, w2t, y16, r2c, 3)
            st2b = _stats_partial(nc, pools, r2c, 3, 4, "2b")

            gstats2 = _stats_allreduce(nc, pools, st2a, st2b, "2")
            a2 = [small.tile([128, 1], F32, tag=f"a2_{c}", name=f"a2_{c}") for c in range(CB)]
            b2 = [small.tile([128, 1], F32, tag=f"b2_{c}", name=f"b2_{c}") for c in range(CB)]
            _bn_affine(nc, pools, gstats2, gb, g_col=4, b_col=6, a_out=a2, b_out=b2)

            # ---- final: out = clip(A2 * y2 + B2 + x, -1, 1), all fp16,
            # load-balanced: affine on ScalarE (6) / VectorE (2), residual add
            # on VectorE, clip on Pool (2) / VectorE (6), output DMA on two
            # queues.
            for i, (n, cib) in enumerate([(n, c) for n in range(NL) for c in range(CB)]):
                yout = youtp.tile([128, H, W], F16, tag="yout", name="yout")
                if i < 6:
                    nc.scalar.activation(
                        out=yout[:], in_=y16[cib][:, n, :, :], func=AF.Identity,
                        scale=a2[cib][:], bias=b2[cib][:])
                else:
                    nc.vector.tensor_scalar(
                        out=yout[:], in0=y16[cib][:, n, :, :],
                        scalar1=a2[cib][:], scalar2=b2[cib][:],
                        op0=ALU.mult, op1=ALU.add)
                nc.vector.tensor_add(yout[:], yout[:], xst[n][cib][:])
                clip_eng = nc.gpsimd if i in (0, 2, 4, 6) else nc.vector
                clip_eng.tensor_scalar(
                    out=yout[:], in0=yout[:], scalar1=1.0, scalar2=-1.0,
                    op0=ALU.min, op1=ALU.max)
                dma_q = nc.sync if i % 2 == 0 else nc.gpsimd
                dma_q.dma_start(
                    out=out_d[n, cib * 128:(cib + 1) * 128, :, :], in_=yout[:])

    nc.compile()
    return nc


def _prep_inputs(x, w1, g1, b1, w2, g2, b2):
    """Host-side sharding + weight layout. Returns per-core input maps."""
    x = np.ascontiguousarray(np.asarray(x, dtype=np.float32))
    # fp16 x halves the HBM load traffic.  The residual add tolerates the
    # ~1e-3 rounding, and sign(x) is made exact by nudging the rare values
    # that would round to fp16 zero up to the smallest normal (sign kept).
    x16 = x.astype(np.float16)
    tiny = (x16 == 0) & (x != 0)
    if tiny.any():
        x16[tiny] = (np.sign(x[tiny]) * 6.104e-5).astype(np.float16)

    # sign(w) pre-transposed to the SBUF layout [ci%128, dy, dx, ci//128, co]
    # (ci = k*128 + p), so the device DMA is fully contiguous; +-1/0 exact
    def prep_w(w):
        wt = np.sign(np.asarray(w, np.float32)).transpose(1, 2, 3, 0)  # ci dy dx co
        wt = wt.reshape(2, 128, 3, 3, C).transpose(1, 2, 3, 0, 4)      # p dy dx k co
        return np.ascontiguousarray(wt).astype(NP_FP8)

    w1t = prep_w(w1)
    w2t = prep_w(w2)
    gb = np.stack(
        [np.asarray(v, np.float32)[c * 128:(c + 1) * 128]
         for v in (g1, b1, g2, b2) for c in range(CB)],
        axis=1,
    )
    # column order: g1_0 g1_1 b1_0 b1_1 g2_0 g2_1 b2_0 b2_1
    gb = np.ascontiguousarray(gb)
    in_maps = []
    for c in range(N_CORES):
        in_maps.append({
            "x16": np.ascontiguousarray(x16[c * NL:(c + 1) * NL]),
            "w1t": w1t,
            "w2t": w2t,
            "gb": gb,
        })
    return in_maps


def run(inputs, trace=False):
    """Run the kernel on 8 cores; returns (full_output, BassKernelResults)."""
    if "nc" not in _CACHE:
        _CACHE["nc"] = build()
    nc = _CACHE["nc"]
    in_maps = _prep_inputs(**inputs)
    res = bass_utils.run_bass_kernel_spmd(
        nc, in_maps, core_ids=list(range(N_CORES)), trace=trace)
    out = np.concatenate(
        [res.results[c]["out"].astype(np.float32) for c in range(N_CORES)], axis=0)
    return out, res


def kernel(**inputs):
    out, _ = run(inputs, trace=False)
    return out


# revision 32
# speedup vs baseline: 1.1462x; 1.1462x over previous
"""Trainium2 Bass kernel for a binarized (1w1a) BasicBlock:

    out1 = hardtanh(BN1(binconv(x, w1)))          # BN in training mode (batch stats)
    out  = hardtanh(BN2(binconv(out1, w2)) + x)   # identity shortcut

binconv(x, w) = conv3x3(sign(x), sign(w), pad=1) * (SCALE / K)

Sharding: data-parallel over batch (4 images per core on 8 cores), weights
replicated.  BN batch statistics (per-channel sum and sum-of-squares) are
combined with tiny cross-core AllReduces.

Implementation notes:
  - sign() values (+-1, 0) are exact in fp8, and the 3x3x256 conv
    accumulates integers |S| <= 2304 in fp32 PSUM, so the convolutions are
    bit-exact at fp8 TensorE rate.  The SCALE/K factor commutes through
    BatchNorm and is folded into eps:  eps_eff = eps / (SCALE/K)^2.
  - conv3x3 = 9 shifted matmuls accumulated in PSUM, with fp8 DoubleRow
    contracting both 128-channel blocks per pass.  Activations live in SBUF
    as [128 ch-pair, 2, n, 58, 64] zero-padded images, so every shift is an
    access-pattern offset.  Each matmul reads 7 padded rows contiguously
    (7 x 64 = 448 columns); the junk columns between output rows are simply
    never read back.  (A 2-D rhs access pattern that skips the padding was
    measured SLOWER: the PE pays a per-row AP overhead.)
  - Weights are pre-transposed on the host to the exact SBUF layout
    ([ci%128, dy, dx, ci//128, co]) so the weight DMA is contiguous.
  - x is loaded from HBM once, on two parallel DMA queues (SP carries the
    first channel block, the Activation queue the second); the staged fp32
    tile feeds both the sign activation (ScalarE -> fp8 conv input) and an
    fp16 residual copy (VectorE) kept resident for the final shortcut add.
  - BN statistics are AllReduced per image: the first three chunks overlap
    the conv (and absorb cross-core launch skew early); only the last
    image's collective latency is exposed at the phase boundary.  All the
    collective plumbing lives on the gpsimd queue so it never blocks the
    x-load or compute queues.
  - The final phase runs in fp16 (exact conv integers + ~1e-3 rounding,
    well inside tolerance), load-balanced: affine on ScalarE (6 blocks) /
    VectorE (2), residual add on VectorE, clip on VectorE / Pool, output
    DMA'd at fp16 on two queues (host converts to fp32).
"""

import numpy as np
import ml_dtypes

import concourse.bass as bass
import concourse.tile as tile
from concourse import bacc, mybir
from concourse import bass_utils

N_CORES = 8
N, C, H, W = 32, 256, 56, 56
NL = N // N_CORES          # images per core
HP = H + 2                 # padded rows (58)
WP = 64                    # padded row pitch: keeps DoubleRow offsets 16B-aligned
IMG = HP * WP              # padded image elements
CB = C // 128              # channel blocks (2)
HT = 7                     # output rows per tile
N_HT = H // HT             # 8 tiles per image
FREE = HT * WP             # 448: 7 padded rows read contiguously
SCALE, K = 1.0, 2
EPS = 1e-5
ALPHA = SCALE / K
EPS_EFF = EPS / (ALPHA * ALPHA)
M_TOT = float(N * H * W)   # BN reduction count (global batch)
HW_ = H * W

F32 = mybir.dt.float32
F16 = mybir.dt.float16
FP8 = mybir.dt.float8e4
NP_FP8 = ml_dtypes.float8_e4m3
AF = mybir.ActivationFunctionType
ALU = mybir.AluOpType
DR = mybir.MatmulPerfMode.DoubleRow

_CACHE = {}


def _conv_img(nc, psum, xbf, wt, y16, recs, n):
    """One binarized conv3x3 over local image n + its stats records."""
    for ht in range(N_HT):
        h0 = ht * HT
        for cob in range(CB):
            # out[h0+r, w] accumulates at pt column r*WP + w + 2 for all
            # nine (dy, dx): the dx shift is applied to the PSUM window so
            # every rhs read stays 16B-aligned (DoubleRow requirement).
            pt = psum.tile([128, FREE + 2], F32, tag="pt", name="pt")
            k = 0
            for dy in range(3):
                for dx in range(3):
                    off = n * IMG + (h0 + dy) * WP
                    nc.tensor.matmul(
                        pt[:, 2 - dx:FREE + 2 - dx],
                        wt[:, dy, dx, :, cob * 128:(cob + 1) * 128],
                        xbf[:, :, off:off + FREE],
                        start=(k == 0),
                        stop=(k == 8),
                        perf_mode=DR,
                    )
                    k += 1
            ptv = pt[:, 2:FREE + 2].rearrange("p (a b) -> p a b", b=WP)
            # fp32 PSUM -> exact integers in fp16
            nc.scalar.activation(
                out=y16[cob][:, n, h0:h0 + HT, :], in_=ptv[:, :, 0:W],
                func=AF.Copy)
            # one Welford record per tile (VectorE, contiguous fp16 read;
            # bn_stats is limited to 512 elements per record)
            yfl = y16[cob][:, n, :, :].rearrange("p a b -> p (a b)")
            nc.vector.bn_stats(
                out=recs[cob][:, n, ht, :],
                in_=yfl[:, h0 * W:(h0 + HT) * W])


def _stats_allreduce(nc, pools, recs, tagp):
    """Convert all local bn_stats records to (sum, sumsq) per channel block
    and AllReduce across the 8 cores.  One collective per conv: the exposed
    boundary cost is launch-skew + one collective latency no matter how the
    stats are chunked, so a single warm AllReduce is optimal.  All DMA +
    trigger plumbing rides the gpsimd queue (nothing else uses it mid-conv).
    Returns the SBUF tile that will hold the global [128, 4] stats.

    NOTE: tile-creation order in this kernel is deliberately frozen — conv
    throughput flips between 189 and 227 ns/matmul depending on the SBUF
    offsets of the weight/xb tiles (sub-bank conflicts between the PE's
    weight-load and rhs streams).  Do not reorder tile allocations."""
    small, dram = pools["small"], pools["dram"]
    rg = [list(range(N_CORES))]
    m_loc = float(NL * HW_)
    st = small.tile([128, 4], F32, tag=f"st{tagp}", name=f"st{tagp}")
    for cob in range(CB):
        mv = small.tile([128, 2], F32, tag=f"mv{tagp}{cob}", name=f"mv{tagp}{cob}")
        rv = recs[cob][:].rearrange("p n t s -> p (n t) s")
        nc.vector.bn_aggr(out=mv[:], in_=rv)
        msq = small.tile([128, 1], F32, tag=f"sq{tagp}{cob}", name=f"sq{tagp}{cob}")
        nc.vector.tensor_scalar_mul(st[:, 2 * cob:2 * cob + 1], mv[:, 0:1], m_loc)
        nc.vector.tensor_mul(msq[:], mv[:, 0:1], mv[:, 0:1])
        nc.vector.tensor_add(msq[:], msq[:], mv[:, 1:2])
        nc.vector.tensor_scalar_mul(st[:, 2 * cob + 1:2 * cob + 2], msq[:], m_loc)
    d_in = dram.tile([128, 4], F32, tag=f"din{tagp}", name=f"din{tagp}")
    d_out = dram.tile([128, 4], F32, tag=f"dout{tagp}", name=f"dout{tagp}")
    nc.gpsimd.dma_start(out=d_in[:], in_=st[:])
    nc.gpsimd.collective_compute(
        "AllReduce", ALU.add, replica_groups=rg,
        ins=[d_in.opt()], outs=[d_out.opt()],
    )
    gsum = small.tile([128, 4], F32, tag=f"gs{tagp}", name=f"gs{tagp}")
    nc.gpsimd.dma_start(out=gsum[:], in_=d_out[:])
    return gsum


def _bn_affine(nc, pools, gstats, gb, g_col, b_col, a_out, b_out):
    """Per-channel-block A/B:  A = g * rsqrt(var + eps_eff),  B = b - mean * A.

    gstats: [128, 4] globally-reduced (sum, sumsq) per channel block
    """
    small = pools["small"]
    epst = pools["epst"]
    for cob in range(CB):
        mean = small.tile([128, 1], F32, tag=f"mean{cob}", name=f"mean{cob}")
        ex2 = small.tile([128, 1], F32, tag=f"ex2{cob}", name=f"ex2{cob}")
        msq = small.tile([128, 1], F32, tag=f"msq{cob}", name=f"msq{cob}")
        var = small.tile([128, 1], F32, tag=f"var{cob}", name=f"var{cob}")
        rstd = small.tile([128, 1], F32, tag=f"rstd{cob}", name=f"rstd{cob}")
        nc.vector.tensor_scalar_mul(mean[:], gstats[:, 2 * cob:2 * cob + 1], 1.0 / M_TOT)
        nc.vector.tensor_scalar_mul(ex2[:], gstats[:, 2 * cob + 1:2 * cob + 2], 1.0 / M_TOT)
        # var = ex2 - mean^2
        nc.vector.tensor_mul(msq[:], mean[:], mean[:])
        nc.vector.tensor_sub(var[:], ex2[:], msq[:])
        # rstd = 1 / sqrt(var + eps_eff)
        nc.scalar.activation(out=rstd[:], in_=var[:], func=AF.Sqrt, bias=epst[:])
        nc.vector.reciprocal(rstd[:], rstd[:])
        # A = g * rstd ; B = b - mean * A
        nc.vector.tensor_mul(a_out[cob][:], gb[:, g_col + cob:g_col + cob + 1], rstd[:])
        nc.vector.tensor_mul(mean[:], mean[:], a_out[cob][:])
        nc.vector.tensor_sub(b_out[cob][:], gb[:, b_col + cob:b_col + cob + 1], mean[:])


def build():
    """Build + compile the per-core Bass program (SPMD, 8 cores)."""
    nc = bacc.Bacc("TRN2", target_bir_lowering=False, debug=False,
                   num_devices=N_CORES)

    x_in = nc.dram_tensor("x16", [NL, C, H, W], F16, kind="ExternalInput").ap()
    w1_in = nc.dram_tensor("w1t", [128, 3, 3, 2, C], FP8, kind="ExternalInput").ap()
    w2_in = nc.dram_tensor("w2t", [128, 3, 3, 2, C], FP8, kind="ExternalInput").ap()
    gb_in = nc.dram_tensor("gb", [128, 8], F32, kind="ExternalInput").ap()
    out_d = nc.dram_tensor("out", [NL, C, H, W], F16, kind="ExternalOutput").ap()

    rg = [list(range(N_CORES))]

    with tile.TileContext(nc) as tc:
        import contextlib
        with contextlib.ExitStack() as ctx:
            consts = ctx.enter_context(tc.tile_pool(name="consts", bufs=1))
            xbp = ctx.enter_context(tc.tile_pool(name="xbp", bufs=1))
            y16p = ctx.enter_context(tc.tile_pool(name="y16p", bufs=1))
            xsp = ctx.enter_context(tc.tile_pool(name="xsp", bufs=1))
            statp = ctx.enter_context(tc.tile_pool(name="statp", bufs=1))
            small = ctx.enter_context(tc.tile_pool(name="small", bufs=1))
            psum = ctx.enter_context(tc.tile_pool(name="psum", bufs=6, space="PSUM"))
            dram = ctx.enter_context(tc.tile_pool(name="dram", bufs=1, space="DRAM"))
            youtp = ctx.enter_context(tc.tile_pool(name="youtp", bufs=8))
            pools = {"small": small, "dram": dram}

            # ---- phase-0 head start: x arrives as fp16 (host-converted,
            # half the HBM traffic) into per-image persistent tiles that
            # double as the residual for the final add.  Image 0's two
            # blocks load on parallel queues (critical path to the first
            # matmul); later images go serially on the SP queue so the DMA
            # traffic is spread thin and does not steal SBUF bandwidth from
            # the PE's rhs reads.
            def phase0(n):
                xs_pair = []
                for cib in range(CB):
                    xs = xsp.tile([128, H, W], F16, tag=f"xs{n}_{cib}",
                                  name=f"xs{n}_{cib}")
                    dma_q = nc.scalar if (n == 0 and cib == 1) else nc.sync
                    dma_q.dma_start(
                        out=xs[:], in_=x_in[n, cib * 128:(cib + 1) * 128, :, :])
                    xs_pair.append(xs)
                return xs_pair

            xs0 = phase0(0)

            epst = small.tile([128, 1], F32, tag="epst", name="epst")
            nc.vector.memset(epst[:], EPS_EFF)
            pools["epst"] = epst

            # ---- dummy AllReduce: absorb first-collective setup cost
            dzero = small.tile([128, 1], F32, tag="dzero", name="dzero")
            nc.vector.memset(dzero[:], 0.0)
            d_in0 = dram.tile([128, 1], F32, tag="d_in0", name="d_in0")
            d_out0 = dram.tile([128, 1], F32, tag="d_out0", name="d_out0")
            nc.gpsimd.dma_start(out=d_in0[:], in_=dzero[:])
            nc.gpsimd.collective_compute(
                "AllReduce", ALU.add, replica_groups=rg,
                ins=[d_in0.opt()], outs=[d_out0.opt()],
            )

            # ---- constants (host already in SBUF layout: contiguous DMAs)
            w1t = consts.tile([128, 3, 3, 2, C], FP8, tag="w1t", name="w1t")
            nc.gpsimd.dma_start(out=w1t[:], in_=w1_in[:])
            w2t = consts.tile([128, 3, 3, 2, C], FP8, tag="w2t", name="w2t")
            nc.gpsimd.dma_start(out=w2t[:], in_=w2_in[:])
            gb = consts.tile([128, 8], F32, tag="gb", name="gb")
            nc.gpsimd.dma_start(out=gb[:], in_=gb_in[:])

            # ---- padded binarized activations (reused: conv1 input, then conv2
            # input).  Rows padded to a 64B pitch so DoubleRow rhs offsets stay
            # 16B-aligned; block stride NL*IMG is 16B-aligned too.
            blk = NL * IMG
            xb = xbp.tile([128, CB, blk], FP8, tag="xb", name="xb")
            xbf = xb[:]
            xbi = [xb[:, cib, :].rearrange(
                "p (n a b) -> p n a b", a=HP, b=WP) for cib in range(CB)]
            # zero only the halo borders + pitch padding (interior is always
            # overwritten by the sign activations before it is read)
            for cib in range(CB):
                nc.vector.memset(xbi[cib][:, :, 0, :], 0.0)
                nc.vector.memset(xbi[cib][:, :, H + 1, :], 0.0)
                nc.vector.memset(xbi[cib][:, :, 1:H + 1, 0:1], 0.0)
                nc.vector.memset(xbi[cib][:, :, 1:H + 1, W + 1:WP], 0.0)

            # ---- conv outputs as exact integers (reused for conv1 then conv2)
            y16 = [y16p.tile([128, NL, H, W], F16, tag=f"y16_{cob}", name=f"y16_{cob}")
                   for cob in range(CB)]

            # ---- bn_stats records (one per row-tile)
            r1c = [statp.tile([128, NL, N_HT, 6], F32, tag=f"r1c{c}", name=f"r1c{c}")
                   for c in range(CB)]
            r2c = [statp.tile([128, NL, N_HT, 6], F32, tag=f"r2c{c}", name=f"r2c{c}")
                   for c in range(CB)]

            # ---- binarize (ScalarE): sign(x) from the fp16 staged tiles
            def binarize(n, xs_pair):
                for cib in range(CB):
                    nc.scalar.activation(
                        out=xbi[cib][:, n, 1:H + 1, 1:W + 1], in_=xs_pair[cib][:],
                        func=AF.Sign)

            # ---- conv1 (loads/signs lead the matmuls)
            xst = [xs0]
            binarize(0, xs0)
            xst.append(phase0(1))
            binarize(1, xst[1])
            _conv_img(nc, psum, xbf, w1t, y16, r1c, 0)
            xst.append(phase0(2))
            binarize(2, xst[2])
            _conv_img(nc, psum, xbf, w1t, y16, r1c, 1)
            xst.append(phase0(3))
            binarize(3, xst[3])
            _conv_img(nc, psum, xbf, w1t, y16, r1c, 2)
            _conv_img(nc, psum, xbf, w1t, y16, r1c, 3)

            gstats1 = _stats_allreduce(nc, pools, r1c, "1")
            a1 = [small.tile([128, 1], F32, tag=f"a1_{c}", name=f"a1_{c}") for c in range(CB)]
            b1 = [small.tile([128, 1], F32, tag=f"b1_{c}", name=f"b1_{c}") for c in range(CB)]
            _bn_affine(nc, pools, gstats1, gb, g_col=0, b_col=2, a_out=a1, b_out=b1)

            # ---- phase 2 + conv2, pipelined per image:
            # out1 = sign(A1 * y1 + B1) written back into the padded buffers.
            # Image 0's signs are split into row halves so conv2's first
            # tiles start after half the sign latency.
            def sign2(n, halves=False):
                rr = [(0, H // 2), (H // 2, H)] if halves else [(0, H)]
                for r0, r1 in rr:
                    for cob in range(CB):
                        nc.scalar.activation(
                            out=xbi[cob][:, n, 1 + r0:1 + r1, 1:W + 1],
                            in_=y16[cob][:, n, r0:r1, :],
                            func=AF.Sign,
                            scale=a1[cob][:],
                            bias=b1[cob][:],
                        )

            sign2(0, halves=True)
            sign2(1)
            _conv_img(nc, psum, xbf, w2t, y16, r2c, 0)
            sign2(2)
            _conv_img(nc, psum, xbf, w2t, y16, r2c, 1)
            sign2(3)
            _conv_img(nc, psum, xbf, w2t, y16, r2c, 2)
            _conv_img(nc, psum, xbf, w2t, y16, r2c, 3)

            gstats2 = _stats_allreduce(nc, pools, r2c, "2")
            a2 = [small.tile([128, 1], F32, tag=f"a2_{c}", name=f"a2_{c}") for c in range(CB)]
            b2 = [small.tile([128, 1], F32, tag=f"b2_{c}", name=f"b2_{c}") for c in range(CB)]
            _bn_affine(nc, pools, gstats2, gb, g_col=4, b_col=6, a_out=a2, b_out=b2)

            # ---- final: out = clip(A2 * y2 + B2 + x, -1, 1), all fp16,
            # load-balanced: affine on ScalarE (6) / VectorE (2), residual add
            # on VectorE, clip on Pool (4) / VectorE (4), output DMA on two
            # queues.  The VectorE-affine blocks are issued first so DVE
            # starts immediately while ScalarE works through its affines.
            blocks = [(n, c) for n in range(NL) for c in range(CB)]
            order = [6, 7, 0, 1, 2, 3, 4, 5]
            for i in order:
                n, cib = blocks[i]
                yout = youtp.tile([128, H, W], F16, tag="yout", name="yout")
                if i < 6:
                    nc.scalar.activation(
                        out=yout[:], in_=y16[cib][:, n, :, :], func=AF.Identity,
                        scale=a2[cib][:], bias=b2[cib][:])
                else:
                    nc.vector.tensor_scalar(
                        out=yout[:], in0=y16[cib][:, n, :, :],
                        scalar1=a2[cib][:], scalar2=b2[cib][:],
                        op0=ALU.mult, op1=ALU.add)
                nc.vector.tensor_add(yout[:], yout[:], xst[n][cib][:])
                clip_eng = nc.gpsimd if i in (0, 2, 4, 6) else nc.vector
                clip_eng.tensor_scalar(
                    out=yout[:], in0=yout[:], scalar1=1.0, scalar2=-1.0,
                    op0=ALU.min, op1=ALU.max)
                dma_q = nc.sync if i % 2 == 0 else nc.gpsimd
                dma_q.dma_start(
                    out=out_d[n, cib * 128:(cib + 1) * 128, :, :], in_=yout[:])

    nc.compile()
    return nc


def _prep_inputs(x, w1, g1, b1, w2, g2, b2):
    """Host-side sharding + weight layout. Returns per-core input maps."""
    x = np.ascontiguousarray(np.asarray(x, dtype=np.float32))
    # fp16 x halves the HBM load traffic.  The residual add tolerates the
    # ~1e-3 rounding, and sign(x) is made exact by nudging the rare values
    # that would round to fp16 zero up to the smallest normal (sign kept).
    x16 = x.astype(np.float16)
    tiny = (x16 == 0) & (x != 0)
    if tiny.any():
        x16[tiny] = (np.sign(x[tiny]) * 6.104e-5).astype(np.float16)

    # sign(w) pre-transposed to the SBUF layout [ci%128, dy, dx, ci//128, co]
    # (ci = k*128 + p), so the device DMA is fully contiguous; +-1/0 exact
    def prep_w(w):
        wt = np.sign(np.asarray(w, np.float32)).transpose(1, 2, 3, 0)  # ci dy dx co
        wt = wt.reshape(2, 128, 3, 3, C).transpose(1, 2, 3, 0, 4)      # p dy dx k co
        return np.ascontiguousarray(wt).astype(NP_FP8)

    w1t = prep_w(w1)
    w2t = prep_w(w2)
    gb = np.stack(
        [np.asarray(v, np.float32)[c * 128:(c + 1) * 128]
         for v in (g1, b1, g2, b2) for c in range(CB)],
        axis=1,
    )
    # column order: g1_0 g1_1 b1_0 b1_1 g2_0 g2_1 b2_0 b2_1
    gb = np.ascontiguousarray(gb)
    in_maps = []
    for c in range(N_CORES):
        in_maps.append({
            "x16": np.ascontiguousarray(x16[c * NL:(c + 1) * NL]),
            "w1t": w1t,
            "w2t": w2t,
            "gb": gb,
        })
    return in_maps


def run(inputs, trace=False):
    """Run the kernel on 8 cores; returns (full_output, BassKernelResults)."""
    if "nc" not in _CACHE:
        _CACHE["nc"] = build()
    nc = _CACHE["nc"]
    in_maps = _prep_inputs(**inputs)
    res = bass_utils.run_bass_kernel_spmd(
        nc, in_maps, core_ids=list(range(N_CORES)), trace=trace)
    out = np.concatenate(
        [res.results[c]["out"].astype(np.float32) for c in range(N_CORES)], axis=0)
    return out, res


def kernel(**inputs):
    out, _ = run(inputs, trace=False)
    return out
